# revision 2
# baseline (speedup 1.0000x reference)
"""Trainium2 Bass kernel for nn_AutoPruneNet — fp8 DoubleRowSwInterleave.

Math (per row r of TB = T*B rows):
    h1 = relu(x @ W1.T + b1)            x: [512], h1: [400]
    h2 = relu(h1 @ W2.T + b2)           h2: [300]
    core = [h2, clip(reward,-1,1), last_action]   [302]
    pl = sigmoid(core @ Wp.T + bp)      [2]  (mu, sigma)
    baseline = core @ Wb.T + bb         [1]
    action = pl0 + pl1 * eps
    out[r] = [pl0, pl1, baseline, action]

Distribution: pure data parallel, TB rows split contiguously across 8 cores
(16384 rows each); weights replicated.

Device strategy vs the bf16 baseline:
  - fc1/fc2 and the main head contraction run as fp8e4 DoubleRow matmuls
    with software-interleaved weights (DoubleRowSwInterleave), halving PE
    multiply cycles and the frame HBM stream; the SW interleave keeps the
    LDWEIGHTS read contiguous (plain DoubleRow gathers, which serializes
    weight loads against matmuls and dominated PE time).
  - Contractions: fc1 512 = 2 pairs of (2x128), h1 padded 400->512 with
    zero weight columns so all chunks are 128 wide (the ISA requires the
    interleaved column count % 16 == 0); fc2 400(+pad) = 2 pairs of
    (2x128); heads 302 = DR pair over h2[0:256] (fp8) + bf16 matmuls for
    h2[256:300] (padded to 48) and [cr, la, 1] (the 1-row folds all three
    head biases into the matmul, so head activations need no bias APs).
  - Engine split: DVE does the four fc1 relus (psum f32 -> fp8); ACT does
    the three fc2 relus + two sigmoids + baseline copy; GPSIMD does the
    SBUF-only epilogue mul/add (it has no PSUM port).
  - clip(reward) is precomputed on host.
"""
import sys
import types

import numpy as np
import ml_dtypes

import concourse.bacc as bacc
import concourse.bass as bass
import concourse.mybir as mybir
import concourse.tile as tile
from concourse.bass import ds, ts
from concourse.bass_utils import run_bass_kernel_spmd


def _install_ntff_hook_shim():
    """Provide the optional antenv.axon_hooks module if the image lacks it,
    so a BASS_TRACE env var in the caller can't crash run_bass_kernel_spmd.
    Registers the real NTFF profile hook when the axon .so supports it."""
    try:
        import antenv.axon_hooks  # noqa: F401
        return
    except Exception:
        pass
    try:
        import antenv
    except Exception:
        return
    mod = types.ModuleType("antenv.axon_hooks")
    state = {"hook": None}
    mod.set_axon_ntff_profile_hook = lambda h: state.__setitem__("hook", h)
    mod.get_axon_ntff_profile_hook = lambda: state["hook"]
    sys.modules["antenv.axon_hooks"] = mod
    antenv.axon_hooks = mod
    try:
        from trn_agent_boot.trn_boot import _ntff_profile_via_ctypes
        mod.set_axon_ntff_profile_hook(
            _ntff_profile_via_ctypes('/opt/axon/libaxon_pjrt.so'))
    except Exception:
        pass


_install_ntff_hook_shim()

BF16 = ml_dtypes.bfloat16
F8 = ml_dtypes.float8_e4m3

N_CORES = 8
T, B, OBS = 64, 2048, 512
H1, H2 = 400, 300
TB = T * B
R = TB // N_CORES       # rows per core
NT = 512                # rows per row-tile (matmul moving dim)
OG = 4                  # row-tiles per output-DMA group

F32 = mybir.dt.float32
BF = mybir.dt.bfloat16
FP8 = mybir.dt.float8e4
AF = mybir.ActivationFunctionType
ALU = mybir.AluOpType
DRS = mybir.MatmulPerfMode.DoubleRowSwInterleave

# fc2 output (h2) chunking: {128, 128, 44->128 padded}
# (DoubleRowSwInterleave LDWEIGHTS requires 128/256 active columns)
M2 = [(0, 128), (128, 128), (256, 128)]


def _swi(wA, wB):
    """Interleave two [P, M] weight blocks into the DoubleRowSwInterleave
    stored layout: per partition [A(M-1), B(M-1), A(M-2), B(M-2), ...]."""
    P, M = wA.shape
    out = np.empty((P, 2 * M), wA.dtype)
    out[:, 0::2] = wA[:, ::-1]
    out[:, 1::2] = wB[:, ::-1]
    return out


def build_bass(rows: int):
    """Build the per-core Bass program for `rows` rows (rows % (NT*OG) == 0)."""
    assert rows % (NT * OG) == 0
    n_tiles = rows // NT

    nc = bacc.Bacc("TRN2", target_bir_lowering=False, debug=False)

    xt_d = nc.dram_tensor("xt", [128, n_tiles, 2, 2, NT], FP8,
                          kind="ExternalInput")
    rw_d = nc.dram_tensor("rw", [3, rows], BF, kind="ExternalInput")
    eps_d = nc.dram_tensor("eps", [1, rows], F32, kind="ExternalInput")
    w1_d = nc.dram_tensor("w1", [128, 2, 4, 256], FP8, kind="ExternalInput")
    w2_d = nc.dram_tensor("w2", [128, 2, 3, 256], FP8, kind="ExternalInput")
    whp_d = nc.dram_tensor("whp", [128, 4, 256], FP8, kind="ExternalInput")
    wh2_d = nc.dram_tensor("wh2", [128, 4, 128], BF, kind="ExternalInput")
    b1_d = nc.dram_tensor("b1", [128, 4], F32, kind="ExternalInput")
    b2_d = nc.dram_tensor("b2", [128, 3], F32, kind="ExternalInput")
    out_d = nc.dram_tensor("out", [4, rows], F32, kind="ExternalOutput")

    with tile.TileContext(nc) as tc:
        with (
            tc.tile_pool(name="w", bufs=1) as wpool,
            tc.tile_pool(name="x", bufs=4) as xpool,
            tc.tile_pool(name="h1", bufs=4) as h1pool,
            tc.tile_pool(name="core", bufs=3) as cpool,
            tc.tile_pool(name="s", bufs=4) as spool,
            tc.tile_pool(name="ob", bufs=2) as opool,
            tc.tile_pool(name="ps1", bufs=4, space="PSUM") as ppool1,
            tc.tile_pool(name="ps2", bufs=2, space="PSUM") as ppool2,
            tc.tile_pool(name="ps3", bufs=2, space="PSUM") as ppool3,
        ):
            w1_sb = wpool.tile([128, 2, 4, 256], FP8, tag="w1")
            nc.scalar.dma_start(w1_sb[:], w1_d[:])
            w2_sb = wpool.tile([128, 2, 3, 256], FP8, tag="w2")
            nc.scalar.dma_start(w2_sb[:], w2_d[:])
            whp_sb = wpool.tile([128, 4, 256], FP8, tag="whp")
            nc.scalar.dma_start(whp_sb[:], whp_d[:])
            wh2_sb = wpool.tile([128, 4, 128], BF, tag="wh2")
            nc.scalar.dma_start(wh2_sb[:], wh2_d[:])
            b1_sb = wpool.tile([128, 4, 1], F32, tag="b1")
            nc.scalar.dma_start(b1_sb[:], b1_d[:])
            b2_sb = wpool.tile([128, 3, 1], F32, tag="b2")
            nc.scalar.dma_start(b2_sb[:], b2_d[:])

            # Software pipeline: head matmuls + epilogue of tile t-1 are
            # emitted between fc1(t) and fc2(t) so the PE has independent
            # work while the fc1 relus land.
            #
            # Heads of OG=4 consecutive tiles accumulate into ONE psum bank
            # at column-shifted rows (tile j: sigma->32+j, mu->36+j,
            # baseline->64+j), so the whole group needs just one sigmoid
            # [32:40], one baseline copy [64:68], one [4,NT] eps-multiply,
            # and one tiny K=8 matmul that recombines action = mu + se.
            obs = {}        # group -> (psh, s68, et36)
            pending = None  # (corep, c2, t) awaiting head matmuls
            epi_q = []      # groups whose head psum is full, epilogue due

            def emit_heads(corep, c2, t):
                g, ti = divmod(t, OG)
                if ti == 0:
                    psh = ppool3.tile([128, NT], F32, tag="ps3")
                    s68 = opool.tile([68, NT], F32, tag="s68")
                    et36 = opool.tile([36, NT], F32, tag="et36")
                    nc.gpsimd.dma_start(et36[32:36, :],
                                        eps_d[:, ts(g, OG * NT)])
                    obs[g] = (psh, s68, et36)
                psh, s68, et36 = obs[g]
                nc.tensor.matmul(psh[:], wh2_sb[:, ti, :], c2[:],
                                 start=(ti == 0), stop=False)
                nc.tensor.matmul(psh[:], whp_sb[:, ti, :], corep[:],
                                 start=False, stop=(ti == OG - 1),
                                 perf_mode=DRS)
                if ti == OG - 1:
                    epi_q.append(g)

            def emit_epilogue(g):
                # deferred ~2 tiles after the group's last head matmul so
                # the PE FIFO never reaches the action matmul before its
                # sigmoid -> pl1-DMA -> gpsimd-mul chain has resolved
                psh, s68, et36 = obs[g]
                gsl = ts(g, OG * NT)
                nc.scalar.activation(s68[32:40, :], psh[32:40, :],
                                     AF.Sigmoid)
                nc.scalar.activation(s68[64:68, :], psh[64:68, :],
                                     AF.Copy)
                nc.gpsimd.dma_start(out_d[1:2, gsl], s68[32:36, :])
                nc.gpsimd.dma_start(out_d[0:1, gsl], s68[36:40, :])
                nc.gpsimd.dma_start(out_d[2:3, gsl], s68[64:68, :])
                # se = pl1 * eps, in place over the sigma rows
                nc.gpsimd.tensor_mul(s68[32:36, :], s68[32:36, :],
                                     et36[32:36, :])
                # action = mu + se: write mu, then a software-DGE DMA
                # accumulates se on top (no PE/ACT involvement)
                nc.gpsimd.dma_start(out_d[3:4, gsl], s68[36:40, :])
                nc.gpsimd.dma_start(out_d[3:4, gsl], s68[32:36, :],
                                    accum_op=ALU.add)
                del obs[g]

            for t in range(n_tiles + 3):
                h1p = None
                if t < n_tiles:
                    xt_t = xpool.tile([128, 2, 2, NT], FP8, tag="xt")
                    nc.sync.dma_start(xt_t[:], xt_d[:, t, :, :, :])

                    # fc1: h1T in 4 chunks of 128 (h1 zero-padded 400->512);
                    # contraction 512 as two DoubleRow pairs of 2x128
                    h1p = [h1pool.tile([128, 2, NT], FP8, tag=f"h1p{c}",
                                       name=f"h1p{c}") for c in (0, 1)]
                    for m in range(4):
                        ps = ppool1.tile([128, NT], F32, tag="ps1")
                        for c in (0, 1):
                            nc.tensor.matmul(
                                ps[:], w1_sb[:, c, m, :], xt_t[:, c, :, :],
                                start=(c == 0), stop=(c == 1), perf_mode=DRS,
                            )
                        # relu(psum + b1) on DVE, fp8 out into the pair tile
                        nc.vector.tensor_scalar(
                            h1p[m // 2][:, m % 2, :], ps[:],
                            b1_sb[:, m, :], 0.0, ALU.add, ALU.max,
                        )

                while epi_q and (t % OG == 2 or t >= n_tiles):
                    emit_epilogue(epi_q.pop(0))
                if pending is not None:
                    emit_heads(*pending)
                    pending = None

                if t < n_tiles:
                    # fc2: h2T chunks {128,128,48}; contraction 512(padded)
                    # as two DoubleRow pairs of 2x128. m=2 first so its relu
                    # (which the head bf16 matmul consumes) lands earliest.
                    corep = cpool.tile([128, 2, NT], FP8, tag="corep")
                    c2 = cpool.tile([128, NT], BF, tag="c2")
                    for m in (2, 0, 1):
                        m0, mw = M2[m]
                        ps2 = ppool2.tile([mw, NT], F32, tag="ps2")
                        for c in (0, 1):
                            nc.tensor.matmul(
                                ps2[:], w2_sb[:, c, m, 0:2 * mw], h1p[c][:],
                                start=(c == 0), stop=(c == 1), perf_mode=DRS,
                            )
                        if m == 0:
                            nc.scalar.activation(corep[:, 0, :], ps2[:],
                                                 AF.Relu,
                                                 bias=b2_sb[0:mw, 0, :])
                        elif m == 1:
                            nc.scalar.activation(corep[:, 1, :], ps2[:],
                                                 AF.Relu,
                                                 bias=b2_sb[0:mw, 1, :])
                        else:
                            nc.scalar.activation(c2[:], ps2[:], AF.Relu,
                                                 bias=b2_sb[0:mw, 2, :])
                            # [cr, la, 1] ride rows 64:67 of c2 (the rest of
                            # rows 44.. is exact zeros); wh2 carries their
                            # head weights there
                            nc.sync.dma_start(c2[64:67, :],
                                              rw_d[:, ts(t, NT)])
                    pending = (corep, c2, t)

    nc.compile()
    return nc


def host_prep(frame, reward, last_action, eps, W1, b1, W2, b2, Wp, bp, Wb, bb,
              rows=R, n_cores=N_CORES):
    """Shard + lay out inputs for the device program. Returns in_maps."""
    n_tiles = rows // NT
    frame = np.asarray(frame, np.float32).reshape(TB, OBS)
    reward = np.asarray(reward, np.float32).reshape(TB)
    la = np.asarray(last_action).reshape(TB).astype(np.float32)
    eps = np.asarray(eps, np.float32).reshape(TB)

    W1 = np.asarray(W1, np.float32)
    W2 = np.asarray(W2, np.float32)
    b1 = np.asarray(b1, np.float32)
    b2 = np.asarray(b2, np.float32)
    Wp = np.asarray(Wp, np.float32)
    bp = np.asarray(bp, np.float32)
    Wb = np.asarray(Wb, np.float32)
    bb = np.asarray(bb, np.float32)

    # fc1: contraction k = pair*256 + sub*128 + p over OBS=512; h1 output
    # padded 400->512 (zero weight columns) so chunks are all 128 wide.
    # fc2: contraction k = pair*256 + sub*128 + p over padded h1.
    W1Tp = np.zeros((OBS, 512), np.float32)
    W1Tp[:, 0:400] = W1.T
    W1T4 = W1Tp.reshape(2, 2, 128, 512)          # [pair, sub, p, m]
    W2Tp = np.zeros((512, 384), np.float32)      # h2 padded 300->384
    W2Tp[0:400, 0:300] = W2.T
    W2T4 = W2Tp.reshape(2, 2, 128, 384)
    w1_h = np.zeros((128, 2, 4, 256), np.float32)
    w2_h = np.zeros((128, 2, 3, 256), np.float32)
    for c in range(2):
        for m in range(4):
            w1_h[:, c, m, :] = _swi(W1T4[c, 0, :, m * 128:(m + 1) * 128],
                                    W1T4[c, 1, :, m * 128:(m + 1) * 128])
        for mi, (m0, mw) in enumerate(M2):
            w2_h[:, c, mi, 0:2 * mw] = _swi(W2T4[c, 0, :, m0:m0 + mw],
                                            W2T4[c, 1, :, m0:m0 + mw])
    w1_h = w1_h.astype(F8)
    w2_h = w2_h.astype(F8)

    # head weights, 4 column-shifted variants (heads of OG=4 tiles share
    # one psum bank): tile j puts sigma at row 32+j, mu at 36+j, baseline
    # at 64+j. Rows follow core = [h2 (300), cr, la] plus an all-ones row
    # that carries the biases.
    whp_h = np.zeros((128, 4, 256), np.float32)
    wh2_h = np.zeros((128, 4, 128), np.float32)
    for j in range(4):
        Whj = np.zeros((303, 128), np.float32)
        Whj[0:302, 36 + j] = Wp[0]
        Whj[302, 36 + j] = bp[0]
        Whj[0:302, 32 + j] = Wp[1]
        Whj[302, 32 + j] = bp[1]
        Whj[0:302, 64 + j] = Wb[0]
        Whj[302, 64 + j] = bb[0]
        whp_h[:, j, :] = _swi(Whj[0:128], Whj[128:256])
        wh2_h[0:44, j, :] = Whj[256:300]
        wh2_h[64:67, j, :] = Whj[300:303]
    whp_h = whp_h.astype(F8)
    wh2_h = wh2_h.astype(BF16)

    b1_h = np.zeros((128, 4), np.float32)
    b1_h[:, 0] = b1[0:128]
    b1_h[:, 1] = b1[128:256]
    b1_h[:, 2] = b1[256:384]
    b1_h[0:16, 3] = b1[384:400]
    b2_h = np.zeros((128, 3), np.float32)
    b2_h[0:128, 0] = b2[0:128]
    b2_h[0:128, 1] = b2[128:256]
    b2_h[0:44, 2] = b2[256:300]

    cr = np.clip(reward, -1.0, 1.0)
    ones = np.ones_like(cr)

    in_maps = []
    for c in range(n_cores):
        sl = slice(c * rows, (c + 1) * rows)
        # [p, tile, pair, sub, n]: feature k = pair*256 + sub*128 + p
        xt = np.ascontiguousarray(
            frame[sl].reshape(n_tiles, NT, 2, 2, 128)
            .transpose(4, 0, 2, 3, 1)).astype(F8)
        rwl = np.stack([cr[sl], la[sl], ones[sl]], axis=0).astype(BF16)
        in_maps.append({
            "xt": xt,
            "rw": rwl,
            "eps": eps[sl].reshape(1, rows),
            "w1": w1_h, "w2": w2_h,
            "whp": whp_h, "wh2": wh2_h,
            "b1": b1_h, "b2": b2_h,
        })
    return in_maps


def assemble_out(per_core_outs):
    """[4, R] per core (rows: pl0, pl1, baseline, action) -> [T, B, 4]."""
    outs = []
    for o in per_core_outs:
        outs.append(np.asarray(o).T.reshape(-1, B, 4))
    return np.ascontiguousarray(
        np.concatenate(outs, axis=0).astype(np.float32))


_NC_CACHE = {}


def kernel(**inputs) -> np.ndarray:
    in_maps = host_prep(**inputs)
    if R not in _NC_CACHE:
        _NC_CACHE[R] = build_bass(R)
    nc = _NC_CACHE[R]
    res = run_bass_kernel_spmd(nc, in_maps, core_ids=list(range(N_CORES)))
    return assemble_out([res.results[c]["out"] for c in range(N_CORES)])
